# revision 16
# baseline (speedup 1.0000x reference)
"""Longformer sliding-window self-attention (B=2, S=4096, D=768, H=12, Dh=64,
one-sided window W=256) on 8 TRN2 NeuronCores.

Sharding: (batch, head-group) - core = b*4 + g handles batch b, heads
[3g, 3g+3). Full-bf16 pipeline per core:

  phase 1: X^T via DMA-xbar transpose (bf16, host pre-converts X), fused
           Q|K projection (W [768,384] bf16, 3 exact 128-row m-tiles),
           bias+copy to bf16 on ACT; V computed directly in [s, dh]
           layout (lhsT = X^T s-tile) into V_aug [s, 3*(64+1)] with a
           ones column (fused softmax denominator).
  phase 2: per 256-query chunk and head, banded scores S^T[k, q] on PE
           (keys on partitions) into a bank-aligned bf16 PSUM layout
           holding only the live (2w+1)-band half-tiles, one Exp on ACT
           per (chunk, head), band-edge masking via DVE 0/1 triangle
           multiplies, O^T = P^T.T @ V_aug accumulated per query-half.
           The ones column yields Z; rows scaled by 1/Z on DVE.
           Chunk emission is software-pipelined two steps behind the
           score matmuls and interleaved with phase 1 so PE stays dense.

kernel() takes full inputs, shards, runs SPMD on cores 0..7, reassembles.
"""
import sys

if '/opt/trn_rl_repo' not in sys.path:
    sys.path.insert(0, '/opt/trn_rl_repo')

import math
from contextlib import ExitStack

import numpy as np
import ml_dtypes

import concourse.bacc as bacc
import concourse.mybir as mybir
import concourse.tile as tile
from concourse.bass_utils import run_bass_kernel_spmd

F32 = mybir.dt.float32
BF16 = mybir.dt.bfloat16

B, S, D = 2, 4096, 768
H, DH, W = 12, 64, 256
HPC = 3              # heads per core
DHC = HPC * DH       # 192 head-dims per core
NCORES = 8
C2 = 256             # query chunk
NCH = S // C2        # 16 chunks
NKT = S // 128       # 32 key tiles
SBLK = 512           # projection s-block
NSB = S // SBLK      # 8 s-blocks
VAW = DH + 1         # 65: V columns + ones column
AluOp = mybir.AluOpType
ActFn = mybir.ActivationFunctionType



def _chunk_layout(ci):
    """Column layout of the banded score tile for chunk ci.

    Halves (edge j=-2 p0 / j=3 p1) sit at cols {0,128}; full 256-col blocks
    start at col 256. All matmul outputs stay within single PSUM banks for
    any 1280-aligned base offset.

    Returns (blocks, ncols, av_blocks, masks):
      blocks: list of (kt, col, width, qoff) score matmuls
      av_blocks[hf]: list of (kt, col) 128-wide P slices for query half hf
      masks: list of (col, which) triangle masks ('ge' or 'le')
    """
    kt0, kt1 = max(0, 2 * ci - 2), min(NKT - 1, 2 * ci + 3)
    fulls = [kt for kt in range(kt0, kt1 + 1) if -2 < kt - 2 * ci < 3]
    blocks, masks = [], []
    av0, av1 = [], []
    col = 0
    if kt0 == 2 * ci - 2:          # j = -2 edge: p0 half only
        blocks.append((kt0, col, 128, 0))
        av0.append((kt0, col))
        masks.append((col, 'ge'))
        col += 128
    if kt1 == 2 * ci + 3:          # j = 3 edge: p1 half only
        blocks.append((kt1, col, 128, 128))
        av1.append((kt1, col))
        masks.append((col, 'le'))
        col += 128
    col = 256
    for kt in fulls:
        j = kt - 2 * ci
        blocks.append((kt, col, 256, 0))
        av0.append((kt, col))
        av1.append((kt, col + 128))
        if j == -1:
            masks.append((col + 128, 'ge'))
        elif j == 2:
            masks.append((col, 'le'))
        col += 256
    # sort AV tiles by kt (accumulation order; first sets start=True)
    av0.sort()
    av1.sort()
    return blocks, col, (av0, av1), masks


def _build_program(use_fmask, use_qmask, add_bv, add_bqk):
    nc = bacc.Bacc("TRN2", num_devices=NCORES)

    x_d = nc.dram_tensor("xt16", (D, S), BF16, kind="ExternalInput").ap()
    wqk_d = nc.dram_tensor("wqk", (D, 2 * DHC), BF16, kind="ExternalInput").ap()
    wv_d = nc.dram_tensor("wv", (D, DHC), BF16, kind="ExternalInput").ap()
    if add_bqk:
        bqk_d = nc.dram_tensor("bqk", (2 * DHC, 1), F32, kind="ExternalInput").ap()
    tge_d = nc.dram_tensor("t_ge", (128, 128), BF16, kind="ExternalInput").ap()
    tle_d = nc.dram_tensor("t_le", (128, 128), BF16, kind="ExternalInput").ap()
    if add_bv:
        bvr_d = nc.dram_tensor("bvrow", (1, DHC), BF16, kind="ExternalInput").ap()
    if use_fmask:
        fmk_d = nc.dram_tensor("fmk", (128, NKT), F32, kind="ExternalInput").ap()
    if use_qmask:
        qmk_d = nc.dram_tensor("qmk", (128, NKT), F32, kind="ExternalInput").ap()
    out_d = nc.dram_tensor("out", (S, DHC), F32, kind="ExternalOutput").ap()

    with tile.TileContext(nc) as tc, ExitStack() as ctx:
        pers = ctx.enter_context(tc.tile_pool(name="pers", bufs=1))

        # persistent constants (wqk loaded after first xT block below)
        wqk = pers.tile([128, 6 * 2 * DHC], BF16, tag="wqk", name="wqk")
        wv = pers.tile([128, 6 * DHC], BF16, tag="wv", name="wv")
        bqk = []
        if add_bqk:
            for m, (c0, msz) in enumerate(
                    ((0, 128), (128, 128), (256, 64), (320, 64))):
                bt = pers.tile([msz, 1], F32, tag=f"bqk{m}", name=f"bqk{m}")
                nc.sync.dma_start(bt[:], bqk_d[c0:c0 + msz, :])
                bqk.append(bt)
        t_ge = pers.tile([128, 128], BF16, tag="t_ge", name="t_ge")
        t_le = pers.tile([128, 128], BF16, tag="t_le", name="t_le")
        nc.sync.dma_start(t_ge[:], tge_d)
        nc.sync.dma_start(t_le[:], tle_d)
        if add_bv:
            bvr = pers.tile([1, DHC], BF16, tag="bvr", name="bvr")
            nc.sync.dma_start(bvr[:], bvr_d)
            ones1 = pers.tile([1, 128], BF16, tag="ones1", name="ones1")
            nc.gpsimd.memset(ones1[:], 1.0)
        if use_fmask:
            fmk = pers.tile([128, NKT], F32, tag="fmk", name="fmk")
            nc.sync.dma_start(fmk[:], fmk_d)
        if use_qmask:
            qmk = pers.tile([128, NKT], F32, tag="qmk", name="qmk")
            nc.sync.dma_start(qmk[:], qmk_d)

        # persistent activations
        xT = [pers.tile([128, S], BF16, tag=f"xT{i}", name=f"xT{i}")
              for i in range(6)]
        qkT = [pers.tile([128 if m < 2 else 64, S], BF16, tag=f"qkT{m}",
                         name=f"qkT{m}") for m in range(4)]
        va = pers.tile([128, NKT * HPC * VAW], BF16, tag="va", name="va")
        va4 = va.rearrange("p (t h c) -> p t h c", h=HPC, c=VAW)
        nc.gpsimd.memset(va4[:, :, :, DH:VAW], 1.0)

        # qkT layout: t0=[q0|q1], t1=[k0|k1], t2=q2, t3=k2 -> q_h and k_h
        # slices always share a partition base (matmul requirement)
        def q_slice(h):
            return (0, 64 * h) if h < 2 else (2, 0)
        def k_slice(h):
            return (1, 64 * h) if h < 2 else (3, 0)

        def emit_xT(sb):
            s0 = sb * SBLK
            for dt in range(6):
                nc.sync.dma_start(
                    xT[dt][:, s0:s0 + SBLK],
                    x_d[dt * 128:(dt + 1) * 128, s0:s0 + SBLK])

        with tc.tile_pool(name="p2s", bufs=1) as p2s, \
             tc.tile_pool(name="pp_pj", bufs=2, space="PSUM") as pjp, \
             tc.tile_pool(name="pp_sc", bufs=1, space="PSUM") as scp, \
             tc.tile_pool(name="pp_av", bufs=1, space="PSUM") as avp:
            scb = scp.tile([128, 2 * 1280], F32, tag="scb", name="scb")
            sc_par = [0]

            def emit_proj_m(sb, m):
                s0 = sb * SBLK
                for m, (c0, msz) in ((m, ((0, 128), (128, 128), (256, 64),
                                          (320, 64))[m]),):
                    pj = pjp.tile([128, SBLK], F32, tag="pj", name="pj")
                    for kt in range(6):
                        nc.tensor.matmul(
                            pj[0:msz, :],
                            wqk[:, kt * 2 * DHC + c0:kt * 2 * DHC + c0 + msz],
                            xT[kt][:, s0:s0 + SBLK],
                            start=(kt == 0), stop=(kt == 5))
                    if add_bqk:
                        nc.vector.tensor_scalar_add(
                            qkT[m][:, s0:s0 + SBLK], pj[0:msz, :], bqk[m][:])
                    else:
                        nc.scalar.activation(qkT[m][:, s0:s0 + SBLK],
                                             pj[0:msz, :], ActFn.Copy)

            def emit_v(st):
                pv = pjp.tile([128, SBLK], F32, tag="pj", name="pv")
                for kt in range(6):
                    nc.tensor.matmul(
                        pv[:, 0:DHC],
                        xT[kt][:, st * 128:(st + 1) * 128],
                        wv[:, kt * DHC:(kt + 1) * DHC],
                        start=(kt == 0), stop=(kt == 5 and not add_bv))
                if add_bv:
                    nc.tensor.matmul(pv[:, 0:DHC], ones1[:], bvr[:],
                                     start=False, stop=True)
                nc.vector.tensor_copy(
                    va4[:, st, :, 0:DH],
                    pv[:, 0:DHC].rearrange("p (h d) -> p h d", h=HPC))

            def attn_front(ci, h):
                """scores -> exp -> masks; returns state for attn_back."""
                blocks, ncols, av_blocks, masks = _chunk_layout(ci)
                mq, rq = q_slice(h)
                mk, rk = k_slice(h)
                par = sc_par[0]
                sc_par[0] ^= 1
                sc = scb[:, par * 1280:(par + 1) * 1280]
                q0 = ci * C2
                for kt, col, wd, qoff in blocks:
                    nc.tensor.matmul(
                        sc[:, col:col + wd],
                        qkT[mk][rk:rk + 64, kt * 128:(kt + 1) * 128],
                        qkT[mq][rq:rq + 64, q0 + qoff:q0 + qoff + wd],
                        start=True, stop=True)
                pt = p2s.tile([128, 1280], BF16, tag="pt", name="pt", bufs=3)
                nc.scalar.activation(pt[:, 0:ncols], sc[:, 0:ncols], ActFn.Exp)
                for col, which in masks:
                    msk = t_ge if which == 'ge' else t_le
                    nc.vector.tensor_tensor(
                        pt[:, col:col + 128], pt[:, col:col + 128], msk[:],
                        op=AluOp.mult)
                if use_fmask:
                    for kt, col, wd, qoff in blocks:
                        nc.vector.tensor_scalar_mul(
                            pt[:, col:col + wd], pt[:, col:col + wd],
                            fmk[:, kt:kt + 1])
                return pt, av_blocks

            def attn_back(ci, h, av, pt, av_blocks):
                for hf in range(2):
                    g = h * 2 + hf
                    lst = av_blocks[hf]
                    for i, (kt, col) in enumerate(lst):
                        nc.tensor.matmul(
                            av[:, g * VAW:g * VAW + VAW],
                            pt[:, col:col + 128], va4[:, kt, h, :],
                            start=(i == 0), stop=(i == len(lst) - 1))

            def epilogue(ci, av):
                av3 = av.rearrange("p (g c) -> p g c", c=VAW)
                rzs = p2s.tile([128, 6], F32, tag="rzs", name="rzs", bufs=3)
                nc.vector.reciprocal(rzs[:], av3[:, :, DH])
                if use_qmask:
                    for g in range(6):
                        nc.vector.tensor_scalar_mul(
                            rzs[:, g:g + 1], rzs[:, g:g + 1],
                            qmk[:, 2 * ci + (g % 2):2 * ci + (g % 2) + 1])
                os_t = [p2s.tile([128, DHC], F32, tag="os", name="os", bufs=4)
                        for _ in range(2)]
                for h in range(HPC):
                    for hf in range(2):
                        g = h * 2 + hf
                        nc.vector.tensor_scalar_mul(
                            os_t[hf][:, h * DH:(h + 1) * DH],
                            av3[:, g, 0:DH], rzs[:, g:g + 1])
                for hf in range(2):
                    qt = 2 * ci + hf
                    nc.gpsimd.dma_start(
                        out_d[qt * 128:(qt + 1) * 128, :], os_t[hf][:])

            # ---- interleaved emission with 2-step software pipeline ----
            ready = {0: [0], 1: [1, 2], 2: [3, 4], 3: [5, 6], 4: [7, 8],
                     5: [9, 10], 6: [11, 12], 7: [13, 14, 15]}
            pending = []          # [(ci, h, av, pt, av_blocks)]
            av_cur = [None]       # av tile for current ci

            def push_step(ci, h):
                pt, av_blocks = attn_front(ci, h)
                if h == 0:
                    av_cur[0] = avp.tile([128, 6 * VAW], F32, tag="av",
                                         name="av")
                pending.append((ci, h, av_cur[0], pt, av_blocks))
                while len(pending) > 2:
                    pop_step()

            def pop_step():
                ci, h, av, pt, av_blocks = pending.pop(0)
                attn_back(ci, h, av, pt, av_blocks)
                if h == HPC - 1:
                    epilogue(ci, av)

            def slot_units(sb):
                # phase-1 work for sb+1, consumed by attn(ready[sb+1]) next
                # slot; V(sb+1) writes must precede any back() that reads them
                if sb + 1 >= NSB:
                    return []
                units = [lambda m=m, sb=sb: emit_proj_m(sb + 1, m)
                         for m in range(4)]
                units += [lambda st=st: emit_v(st)
                          for st in range(4 * (sb + 1), 4 * (sb + 1) + 4)]
                return units

            emit_xT(0)
            nc.sync.dma_start(wqk[:], wqk_d.rearrange("(a p) n -> p a n", p=128))
            emit_xT(1)
            nc.sync.dma_start(wv[:], wv_d.rearrange("(a p) n -> p a n", p=128))
            for m in range(4):
                emit_proj_m(0, m)
            for st in range(4):
                emit_v(st)
            for sb in range(NSB):
                if sb + 2 <= NSB - 1:
                    emit_xT(sb + 2)
                p1 = slot_units(sb)
                at = [(ci, h) for ci in ready[sb] for h in range(HPC)]
                k = 0
                while k < max(len(p1), len(at)):
                    if k < len(at):
                        push_step(*at[k])
                    if k < len(p1):
                        p1[k]()
                    k += 1
            while pending:
                pop_step()

    nc.compile()
    return nc


_prog_cache = {}


def _get_program(use_fmask, use_qmask, add_bv, add_bqk):
    key = (use_fmask, use_qmask, add_bv, add_bqk)
    if key not in _prog_cache:
        _prog_cache[key] = _build_program(use_fmask, use_qmask, add_bv, add_bqk)
    return _prog_cache[key]


def _host_constants():
    kl = np.arange(128)[:, None]
    ql = np.arange(128)[None, :]
    t_ge = (kl >= ql).astype(ml_dtypes.bfloat16)
    t_le = (kl <= ql).astype(ml_dtypes.bfloat16)
    return t_ge, t_le


def kernel(hidden_states, attention_mask, is_index_masked, Wq, bq, Wk, bk, Wv, bv,
           trace=False):
    hidden_states = np.asarray(hidden_states, dtype=np.float32)
    attention_mask = np.asarray(attention_mask, dtype=np.float32)
    is_index_masked = np.asarray(is_index_masked)
    Wq = np.asarray(Wq, dtype=np.float32)
    Wk = np.asarray(Wk, dtype=np.float32)
    Wv = np.asarray(Wv, dtype=np.float32)
    bq = np.asarray(bq, dtype=np.float32)
    bk = np.asarray(bk, dtype=np.float32)
    bv = np.asarray(bv, dtype=np.float32)

    use_fmask = bool(np.any(attention_mask != 0))
    use_qmask = bool(np.any(is_index_masked))
    add_bv = bool(np.any(bv != 0))
    add_bqk = bool(np.any(bq != 0) or np.any(bk != 0))
    nc = _get_program(use_fmask, use_qmask, add_bv, add_bqk)

    scale = 1.0 / math.sqrt(DH)
    t_ge, t_le = _host_constants()
    xt16 = [np.ascontiguousarray(hidden_states[b].astype(ml_dtypes.bfloat16).T)
            for b in range(B)]

    in_maps = []
    for cid in range(NCORES):
        b = cid // 4
        h0 = HPC * (cid % 4)
        c0, c1 = h0 * DH, (h0 + HPC) * DH
        wqk = np.concatenate([
            Wq[:, c0:c0 + 128] * scale, Wk[:, c0:c0 + 128],
            Wq[:, c0 + 128:c1] * scale, Wk[:, c0 + 128:c1]], axis=1)
        m = {
            "xt16": xt16[b],
            "wqk": np.ascontiguousarray(wqk.astype(ml_dtypes.bfloat16)),
            "wv": np.ascontiguousarray(Wv[:, c0:c1].astype(ml_dtypes.bfloat16)),
            "t_ge": t_ge,
            "t_le": t_le,
        }
        if add_bqk:
            bqk = np.concatenate([
                bq[c0:c0 + 128] * scale, bk[c0:c0 + 128],
                bq[c0 + 128:c1] * scale, bk[c0 + 128:c1]])
            m["bqk"] = np.ascontiguousarray(bqk.reshape(2 * DHC, 1))
        if add_bv:
            m["bvrow"] = np.ascontiguousarray(
                bv[c0:c1].astype(ml_dtypes.bfloat16).reshape(1, DHC))
        if use_fmask:
            fac = (attention_mask[b] == 0).astype(np.float32)  # keep-factor
            m["fmk"] = np.ascontiguousarray(fac.reshape(NKT, 128).T)
        if use_qmask:
            keep = (~is_index_masked[b]).astype(np.float32)
            m["qmk"] = np.ascontiguousarray(keep.reshape(NKT, 128).T)
        in_maps.append(m)

    res = run_bass_kernel_spmd(nc, in_maps, core_ids=list(range(NCORES)),
                               trace=trace)
    out = np.empty((B, S, D), dtype=np.float32)
    for cid in range(NCORES):
        b = cid // 4
        h0 = HPC * (cid % 4)
        out[b, :, h0 * DH:(h0 + HPC) * DH] = res.results[cid]["out"]
    if trace:
        return out, res
    return out


# revision 17
# speedup vs baseline: 1.0230x; 1.0230x over previous
"""Longformer sliding-window self-attention (B=2, S=4096, D=768, H=12, Dh=64,
one-sided window W=256) on 8 TRN2 NeuronCores.

Sharding: (batch, head-group) - core = b*4 + g handles batch b, heads
[3g, 3g+3). Full-bf16 pipeline per core:

  phase 1: X^T via DMA-xbar transpose (bf16, host pre-converts X), fused
           Q|K projection (W [768,384] bf16, 3 exact 128-row m-tiles),
           bias+copy to bf16 on ACT; V computed directly in [s, dh]
           layout (lhsT = X^T s-tile) into V_aug [s, 3*(64+1)] with a
           ones column (fused softmax denominator).
  phase 2: per 256-query chunk and head, banded scores S^T[k, q] on PE
           (keys on partitions) into a bank-aligned bf16 PSUM layout
           holding only the live (2w+1)-band half-tiles, one Exp on ACT
           per (chunk, head), band-edge masking via DVE 0/1 triangle
           multiplies, O^T = P^T.T @ V_aug accumulated per query-half.
           The ones column yields Z; rows scaled by 1/Z on DVE.
           Chunk emission is software-pipelined two steps behind the
           score matmuls and interleaved with phase 1 so PE stays dense.

kernel() takes full inputs, shards, runs SPMD on cores 0..7, reassembles.
"""
import sys

if '/opt/trn_rl_repo' not in sys.path:
    sys.path.insert(0, '/opt/trn_rl_repo')

import math
from contextlib import ExitStack

import numpy as np
import ml_dtypes

import concourse.bacc as bacc
import concourse.mybir as mybir
import concourse.tile as tile
from concourse.bass_utils import run_bass_kernel_spmd

F32 = mybir.dt.float32
BF16 = mybir.dt.bfloat16

B, S, D = 2, 4096, 768
H, DH, W = 12, 64, 256
HPC = 3              # heads per core
DHC = HPC * DH       # 192 head-dims per core
NCORES = 8
C2 = 256             # query chunk
NCH = S // C2        # 16 chunks
NKT = S // 128       # 32 key tiles
SBLK = 512           # projection s-block
NSB = S // SBLK      # 8 s-blocks
VAW = DH + 1         # 65: V columns + ones column
AluOp = mybir.AluOpType
ActFn = mybir.ActivationFunctionType



def _chunk_layout(ci):
    """Column layout of the banded score tile for chunk ci.

    Halves (edge j=-2 p0 / j=3 p1) sit at cols {0,128}; full 256-col blocks
    start at col 256. All matmul outputs stay within single PSUM banks for
    any 1280-aligned base offset.

    Returns (blocks, ncols, av_blocks, masks):
      blocks: list of (kt, col, width, qoff) score matmuls
      av_blocks[hf]: list of (kt, col) 128-wide P slices for query half hf
      masks: list of (col, which) triangle masks ('ge' or 'le')
    """
    kt0, kt1 = max(0, 2 * ci - 2), min(NKT - 1, 2 * ci + 3)
    fulls = [kt for kt in range(kt0, kt1 + 1) if -2 < kt - 2 * ci < 3]
    blocks, masks = [], []
    av0, av1 = [], []
    col = 0
    if kt0 == 2 * ci - 2:          # j = -2 edge: p0 half only
        blocks.append((kt0, col, 128, 0))
        av0.append((kt0, col))
        masks.append((col, 'ge'))
        col += 128
    if kt1 == 2 * ci + 3:          # j = 3 edge: p1 half only
        blocks.append((kt1, col, 128, 128))
        av1.append((kt1, col))
        masks.append((col, 'le'))
        col += 128
    col = 256
    for kt in fulls:
        j = kt - 2 * ci
        blocks.append((kt, col, 256, 0))
        av0.append((kt, col))
        av1.append((kt, col + 128))
        if j == -1:
            masks.append((col + 128, 'ge'))
        elif j == 2:
            masks.append((col, 'le'))
        col += 256
    # sort AV tiles by kt (accumulation order; first sets start=True)
    av0.sort()
    av1.sort()
    return blocks, col, (av0, av1), masks


def _build_program(use_fmask, use_qmask, add_bv, add_bqk):
    nc = bacc.Bacc("TRN2", num_devices=NCORES)

    x_d = nc.dram_tensor("xt16", (D, S), BF16, kind="ExternalInput").ap()
    wqk_d = nc.dram_tensor("wqk", (D, 2 * DHC), BF16, kind="ExternalInput").ap()
    wv_d = nc.dram_tensor("wv", (D, DHC), BF16, kind="ExternalInput").ap()
    if add_bqk:
        bqk_d = nc.dram_tensor("bqk", (2 * DHC, 1), F32, kind="ExternalInput").ap()
    tge_d = nc.dram_tensor("t_ge", (128, 128), BF16, kind="ExternalInput").ap()
    tle_d = nc.dram_tensor("t_le", (128, 128), BF16, kind="ExternalInput").ap()
    if add_bv:
        bvr_d = nc.dram_tensor("bvrow", (1, DHC), BF16, kind="ExternalInput").ap()
    if use_fmask:
        fmk_d = nc.dram_tensor("fmk", (128, NKT), F32, kind="ExternalInput").ap()
    if use_qmask:
        qmk_d = nc.dram_tensor("qmk", (128, NKT), F32, kind="ExternalInput").ap()
    out_d = nc.dram_tensor("out", (S, DHC), F32, kind="ExternalOutput").ap()

    with tile.TileContext(nc) as tc, ExitStack() as ctx:
        pers = ctx.enter_context(tc.tile_pool(name="pers", bufs=1))

        # persistent constants (wqk loaded after first xT block below)
        wqk = pers.tile([128, 6 * 2 * DHC], BF16, tag="wqk", name="wqk")
        wv = pers.tile([128, 6 * DHC], BF16, tag="wv", name="wv")
        bqk = []
        if add_bqk:
            for m, (c0, msz) in enumerate(
                    ((0, 128), (128, 128), (256, 64), (320, 64))):
                bt = pers.tile([msz, 1], F32, tag=f"bqk{m}", name=f"bqk{m}")
                nc.sync.dma_start(bt[:], bqk_d[c0:c0 + msz, :])
                bqk.append(bt)
        t_ge = pers.tile([128, 128], BF16, tag="t_ge", name="t_ge")
        t_le = pers.tile([128, 128], BF16, tag="t_le", name="t_le")
        if add_bv:
            bvr = pers.tile([1, DHC], BF16, tag="bvr", name="bvr")
            nc.sync.dma_start(bvr[:], bvr_d)
            ones1 = pers.tile([1, 128], BF16, tag="ones1", name="ones1")
            nc.gpsimd.memset(ones1[:], 1.0)
        if use_fmask:
            fmk = pers.tile([128, NKT], F32, tag="fmk", name="fmk")
            nc.sync.dma_start(fmk[:], fmk_d)
        if use_qmask:
            qmk = pers.tile([128, NKT], F32, tag="qmk", name="qmk")
            nc.sync.dma_start(qmk[:], qmk_d)

        # persistent activations
        xT = [pers.tile([128, S], BF16, tag=f"xT{i}", name=f"xT{i}")
              for i in range(6)]
        qkT = [pers.tile([128 if m < 2 else 64, S], BF16, tag=f"qkT{m}",
                         name=f"qkT{m}") for m in range(4)]
        va = pers.tile([128, NKT * HPC * VAW], BF16, tag="va", name="va")
        va4 = va.rearrange("p (t h c) -> p t h c", h=HPC, c=VAW)
        nc.gpsimd.memset(va4[:, :, :, DH:VAW], 1.0)

        # qkT layout: t0=[q0|q1], t1=[k0|k1], t2=q2, t3=k2 -> q_h and k_h
        # slices always share a partition base (matmul requirement)
        def q_slice(h):
            return (0, 64 * h) if h < 2 else (2, 0)
        def k_slice(h):
            return (1, 64 * h) if h < 2 else (3, 0)

        def emit_xT(sb):
            s0 = sb * SBLK
            for dt in range(6):
                nc.sync.dma_start(
                    xT[dt][:, s0:s0 + SBLK],
                    x_d[dt * 128:(dt + 1) * 128, s0:s0 + SBLK])

        with tc.tile_pool(name="p2s", bufs=1) as p2s, \
             tc.tile_pool(name="pp_pj", bufs=2, space="PSUM") as pjp, \
             tc.tile_pool(name="pp_sc", bufs=1, space="PSUM") as scp, \
             tc.tile_pool(name="pp_av", bufs=1, space="PSUM") as avp:
            scb = scp.tile([128, 2 * 1280], F32, tag="scb", name="scb")
            sc_par = [0]

            def emit_proj_m(sb, m):
                s0 = sb * SBLK
                for m, (c0, msz) in ((m, ((0, 128), (128, 128), (256, 64),
                                          (320, 64))[m]),):
                    pj = pjp.tile([128, SBLK], F32, tag="pj", name="pj")
                    for kt in range(6):
                        nc.tensor.matmul(
                            pj[0:msz, :],
                            wqk[:, kt * 2 * DHC + c0:kt * 2 * DHC + c0 + msz],
                            xT[kt][:, s0:s0 + SBLK],
                            start=(kt == 0), stop=(kt == 5))
                    if add_bqk:
                        nc.vector.tensor_scalar_add(
                            qkT[m][:, s0:s0 + SBLK], pj[0:msz, :], bqk[m][:])
                    else:
                        nc.scalar.activation(qkT[m][:, s0:s0 + SBLK],
                                             pj[0:msz, :], ActFn.Copy)

            def emit_v(st):
                pv = pjp.tile([128, SBLK], F32, tag="pj", name="pv")
                for kt in range(6):
                    nc.tensor.matmul(
                        pv[:, 0:DHC],
                        xT[kt][:, st * 128:(st + 1) * 128],
                        wv[:, kt * DHC:(kt + 1) * DHC],
                        start=(kt == 0), stop=(kt == 5 and not add_bv))
                if add_bv:
                    nc.tensor.matmul(pv[:, 0:DHC], ones1[:], bvr[:],
                                     start=False, stop=True)
                nc.vector.tensor_copy(
                    va4[:, st, :, 0:DH],
                    pv[:, 0:DHC].rearrange("p (h d) -> p h d", h=HPC))

            def attn_front(ci, h):
                """scores -> exp -> masks; returns state for attn_back."""
                blocks, ncols, av_blocks, masks = _chunk_layout(ci)
                mq, rq = q_slice(h)
                mk, rk = k_slice(h)
                par = sc_par[0]
                sc_par[0] ^= 1
                sc = scb[:, par * 1280:(par + 1) * 1280]
                q0 = ci * C2
                for kt, col, wd, qoff in blocks:
                    nc.tensor.matmul(
                        sc[:, col:col + wd],
                        qkT[mk][rk:rk + 64, kt * 128:(kt + 1) * 128],
                        qkT[mq][rq:rq + 64, q0 + qoff:q0 + qoff + wd],
                        start=True, stop=True)
                pt = p2s.tile([128, 1280], BF16, tag="pt", name="pt", bufs=4)
                nc.scalar.activation(pt[:, 0:ncols], sc[:, 0:ncols], ActFn.Exp)
                for col, which in masks:
                    msk = t_ge if which == 'ge' else t_le
                    nc.vector.tensor_tensor(
                        pt[:, col:col + 128], pt[:, col:col + 128], msk[:],
                        op=AluOp.mult)
                if use_fmask:
                    for kt, col, wd, qoff in blocks:
                        nc.vector.tensor_scalar_mul(
                            pt[:, col:col + wd], pt[:, col:col + wd],
                            fmk[:, kt:kt + 1])
                return pt, av_blocks

            def attn_back(ci, h, av, pt, av_blocks):
                for hf in range(2):
                    g = h * 2 + hf
                    lst = av_blocks[hf]
                    for i, (kt, col) in enumerate(lst):
                        nc.tensor.matmul(
                            av[:, g * VAW:g * VAW + VAW],
                            pt[:, col:col + 128], va4[:, kt, h, :],
                            start=(i == 0), stop=(i == len(lst) - 1))

            def epilogue(ci, av):
                av3 = av.rearrange("p (g c) -> p g c", c=VAW)
                rzs = p2s.tile([128, 6], F32, tag="rzs", name="rzs", bufs=3)
                nc.vector.reciprocal(rzs[:], av3[:, :, DH])
                if use_qmask:
                    for g in range(6):
                        nc.vector.tensor_scalar_mul(
                            rzs[:, g:g + 1], rzs[:, g:g + 1],
                            qmk[:, 2 * ci + (g % 2):2 * ci + (g % 2) + 1])
                os_t = [p2s.tile([128, DHC], F32, tag="os", name="os", bufs=4)
                        for _ in range(2)]
                for h in range(HPC):
                    for hf in range(2):
                        g = h * 2 + hf
                        nc.vector.tensor_scalar_mul(
                            os_t[hf][:, h * DH:(h + 1) * DH],
                            av3[:, g, 0:DH], rzs[:, g:g + 1])
                for hf in range(2):
                    qt = 2 * ci + hf
                    nc.gpsimd.dma_start(
                        out_d[qt * 128:(qt + 1) * 128, :], os_t[hf][:])

            # ---- interleaved emission with 2-step software pipeline ----
            ready = {0: [0], 1: [1, 2], 2: [3, 4], 3: [5, 6], 4: [7, 8],
                     5: [9, 10], 6: [11, 12], 7: [13, 14, 15]}
            pending = []          # [(ci, h, av, pt, av_blocks)]
            av_cur = [None]       # av tile for current ci

            def push_step(ci, h):
                pt, av_blocks = attn_front(ci, h)
                if h == 0:
                    av_cur[0] = avp.tile([128, 6 * VAW], F32, tag="av",
                                         name="av")
                pending.append((ci, h, av_cur[0], pt, av_blocks))
                while len(pending) > 3:
                    pop_step()

            def pop_step():
                ci, h, av, pt, av_blocks = pending.pop(0)
                attn_back(ci, h, av, pt, av_blocks)
                if h == HPC - 1:
                    epilogue(ci, av)

            def slot_units(sb):
                # phase-1 work for sb+1, consumed by attn(ready[sb+1]) next
                # slot; V(sb+1) writes must precede any back() that reads them
                if sb + 1 >= NSB:
                    return []
                units = [lambda m=m, sb=sb: emit_proj_m(sb + 1, m)
                         for m in range(4)]
                units += [lambda st=st: emit_v(st)
                          for st in range(4 * (sb + 1), 4 * (sb + 1) + 4)]
                return units

            emit_xT(0)
            nc.sync.dma_start(wqk[:], wqk_d.rearrange("(a p) n -> p a n", p=128))
            emit_xT(1)
            nc.sync.dma_start(wv[:], wv_d.rearrange("(a p) n -> p a n", p=128))
            nc.sync.dma_start(t_ge[:], tge_d)
            nc.sync.dma_start(t_le[:], tle_d)
            for m in range(4):
                emit_proj_m(0, m)
            for st in range(4):
                emit_v(st)
            for sb in range(NSB):
                if sb + 2 <= NSB - 1:
                    emit_xT(sb + 2)
                p1 = slot_units(sb)
                at = [(ci, h) for ci in ready[sb] for h in range(HPC)]
                k = 0
                while k < max(len(p1), len(at)):
                    if k < len(at):
                        push_step(*at[k])
                    if k < len(p1):
                        p1[k]()
                    k += 1
            while pending:
                pop_step()

    nc.compile()
    return nc


_prog_cache = {}


def _get_program(use_fmask, use_qmask, add_bv, add_bqk):
    key = (use_fmask, use_qmask, add_bv, add_bqk)
    if key not in _prog_cache:
        _prog_cache[key] = _build_program(use_fmask, use_qmask, add_bv, add_bqk)
    return _prog_cache[key]


def _host_constants():
    kl = np.arange(128)[:, None]
    ql = np.arange(128)[None, :]
    t_ge = (kl >= ql).astype(ml_dtypes.bfloat16)
    t_le = (kl <= ql).astype(ml_dtypes.bfloat16)
    return t_ge, t_le


def kernel(hidden_states, attention_mask, is_index_masked, Wq, bq, Wk, bk, Wv, bv,
           trace=False):
    hidden_states = np.asarray(hidden_states, dtype=np.float32)
    attention_mask = np.asarray(attention_mask, dtype=np.float32)
    is_index_masked = np.asarray(is_index_masked)
    Wq = np.asarray(Wq, dtype=np.float32)
    Wk = np.asarray(Wk, dtype=np.float32)
    Wv = np.asarray(Wv, dtype=np.float32)
    bq = np.asarray(bq, dtype=np.float32)
    bk = np.asarray(bk, dtype=np.float32)
    bv = np.asarray(bv, dtype=np.float32)

    use_fmask = bool(np.any(attention_mask != 0))
    use_qmask = bool(np.any(is_index_masked))
    add_bv = bool(np.any(bv != 0))
    add_bqk = bool(np.any(bq != 0) or np.any(bk != 0))
    nc = _get_program(use_fmask, use_qmask, add_bv, add_bqk)

    scale = 1.0 / math.sqrt(DH)
    t_ge, t_le = _host_constants()
    xt16 = [np.ascontiguousarray(hidden_states[b].astype(ml_dtypes.bfloat16).T)
            for b in range(B)]

    in_maps = []
    for cid in range(NCORES):
        b = cid // 4
        h0 = HPC * (cid % 4)
        c0, c1 = h0 * DH, (h0 + HPC) * DH
        wqk = np.concatenate([
            Wq[:, c0:c0 + 128] * scale, Wk[:, c0:c0 + 128],
            Wq[:, c0 + 128:c1] * scale, Wk[:, c0 + 128:c1]], axis=1)
        m = {
            "xt16": xt16[b],
            "wqk": np.ascontiguousarray(wqk.astype(ml_dtypes.bfloat16)),
            "wv": np.ascontiguousarray(Wv[:, c0:c1].astype(ml_dtypes.bfloat16)),
            "t_ge": t_ge,
            "t_le": t_le,
        }
        if add_bqk:
            bqk = np.concatenate([
                bq[c0:c0 + 128] * scale, bk[c0:c0 + 128],
                bq[c0 + 128:c1] * scale, bk[c0 + 128:c1]])
            m["bqk"] = np.ascontiguousarray(bqk.reshape(2 * DHC, 1))
        if add_bv:
            m["bvrow"] = np.ascontiguousarray(
                bv[c0:c1].astype(ml_dtypes.bfloat16).reshape(1, DHC))
        if use_fmask:
            fac = (attention_mask[b] == 0).astype(np.float32)  # keep-factor
            m["fmk"] = np.ascontiguousarray(fac.reshape(NKT, 128).T)
        if use_qmask:
            keep = (~is_index_masked[b]).astype(np.float32)
            m["qmk"] = np.ascontiguousarray(keep.reshape(NKT, 128).T)
        in_maps.append(m)

    res = run_bass_kernel_spmd(nc, in_maps, core_ids=list(range(NCORES)),
                               trace=trace)
    out = np.empty((B, S, D), dtype=np.float32)
    for cid in range(NCORES):
        b = cid // 4
        h0 = HPC * (cid % 4)
        out[b, :, h0 * DH:(h0 + HPC) * DH] = res.results[cid]["out"]
    if trace:
        return out, res
    return out


# revision 28
# speedup vs baseline: 1.2145x; 1.1872x over previous
"""Longformer sliding-window self-attention (B=2, S=4096, D=768, H=12, Dh=64,
one-sided window W=256) on 8 TRN2 NeuronCores.

Sharding: (batch, head-group) - core = b*4 + g handles batch b, heads
[3g, 3g+3). Full-bf16 pipeline per core (rel err ~4e-3 vs f32 ref):

  phase 1: X^T arrives pre-transposed from the host (bf16), loaded with
           one merged DMA per 512-seq block; Q|K projection with weights
           packed [q0q1|k0k1|q2|k2] so each head's q/k slices share a
           partition base (matmul operand requirement); V computed
           directly in [s, dh] layout (lhsT = X^T s-tile, rhs = Wv) into
           V_aug [s, 3*(64+1)] with a ones column (fused softmax
           denominator). PSUM evacuations on DVE.
  phase 2: per 256-query chunk, banded scores S^T[k, q] on PE (keys on
           partitions) into a 2x1280-col f32 PSUM ping-pong holding only
           the live band half-tiles (bank-aligned, halves-first layout);
           heads 0/1 (partition bases 0/64) emit score matmuls
           interleaved so they run concurrently in disjoint PE row
           groups and share one fused 2560-col Exp on ACT; band-edge
           masking via DVE triangle multiplies (pairs fused with custom
           strided APs); O^T = P^T.T @ V_aug accumulated per query-half.
           The ones column yields Z; rows scaled by 1/Z with broadcast
           DVE multiplies. Attention steps are software-pipelined three
           steps deep and weight-interleaved with phase-1 units so PE
           stays dense and the HAM clock stays warm.

kernel() takes full inputs, shards, runs SPMD on cores 0..7, reassembles.
"""
import sys

if '/opt/trn_rl_repo' not in sys.path:
    sys.path.insert(0, '/opt/trn_rl_repo')

import math
from contextlib import ExitStack

import numpy as np
import ml_dtypes

import concourse.bacc as bacc
import concourse.mybir as mybir
from concourse.ap import AP
import concourse.tile as tile
from concourse.bass_utils import run_bass_kernel_spmd

F32 = mybir.dt.float32
BF16 = mybir.dt.bfloat16

B, S, D = 2, 4096, 768
H, DH, W = 12, 64, 256
HPC = 3              # heads per core
DHC = HPC * DH       # 192 head-dims per core
NCORES = 8
C2 = 256             # query chunk
NCH = S // C2        # 16 chunks
NKT = S // 128       # 32 key tiles
SBLK = 512           # projection s-block
NSB = S // SBLK      # 8 s-blocks
VAW = DH + 1         # 65: V columns + ones column
AluOp = mybir.AluOpType
ActFn = mybir.ActivationFunctionType



def _chunk_layout(ci):
    """Column layout of the banded score tile for chunk ci.

    Halves (edge j=-2 p0 / j=3 p1) sit at cols {0,128}; full 256-col blocks
    start at col 256. All matmul outputs stay within single PSUM banks for
    any 1280-aligned base offset.

    Returns (blocks, ncols, av_blocks, masks):
      blocks: list of (kt, col, width, qoff) score matmuls
      av_blocks[hf]: list of (kt, col) 128-wide P slices for query half hf
      masks: list of (col, which) triangle masks ('ge' or 'le')
    """
    kt0, kt1 = max(0, 2 * ci - 2), min(NKT - 1, 2 * ci + 3)
    fulls = [kt for kt in range(kt0, kt1 + 1) if -2 < kt - 2 * ci < 3]
    blocks, masks = [], []
    av0, av1 = [], []
    col = 0
    if kt0 == 2 * ci - 2:          # j = -2 edge: p0 half only
        blocks.append((kt0, col, 128, 0))
        av0.append((kt0, col))
        masks.append((col, 'ge'))
        col += 128
    if kt1 == 2 * ci + 3:          # j = 3 edge: p1 half only
        blocks.append((kt1, col, 128, 128))
        av1.append((kt1, col))
        masks.append((col, 'le'))
        col += 128
    col = 256
    for kt in fulls:
        j = kt - 2 * ci
        blocks.append((kt, col, 256, 0))
        av0.append((kt, col))
        av1.append((kt, col + 128))
        if j == -1:
            masks.append((col + 128, 'ge'))
        elif j == 2:
            masks.append((col, 'le'))
        col += 256
    # sort AV tiles by kt (accumulation order; first sets start=True)
    av0.sort()
    av1.sort()
    return blocks, col, (av0, av1), masks


def _build_program(use_fmask, use_qmask, add_bv, add_bqk):
    nc = bacc.Bacc("TRN2", num_devices=NCORES)

    x_d = nc.dram_tensor("xt16", (D, S), BF16, kind="ExternalInput").ap()
    wqk_d = nc.dram_tensor("wqk", (D, 2 * DHC), BF16, kind="ExternalInput").ap()
    wv_d = nc.dram_tensor("wv", (D, DHC), BF16, kind="ExternalInput").ap()
    if add_bqk:
        bqk_d = nc.dram_tensor("bqk", (2 * DHC, 1), F32, kind="ExternalInput").ap()
    tge_d = nc.dram_tensor("t_ge", (128, 128), BF16, kind="ExternalInput").ap()
    tle_d = nc.dram_tensor("t_le", (128, 128), BF16, kind="ExternalInput").ap()
    if add_bv:
        bvr_d = nc.dram_tensor("bvrow", (1, DHC), BF16, kind="ExternalInput").ap()
    if use_fmask:
        fmk_d = nc.dram_tensor("fmk", (128, NKT), F32, kind="ExternalInput").ap()
    if use_qmask:
        qmk_d = nc.dram_tensor("qmk", (128, NKT), F32, kind="ExternalInput").ap()
    out_d = nc.dram_tensor("out", (S, DHC), F32, kind="ExternalOutput").ap()

    with tile.TileContext(nc) as tc, ExitStack() as ctx:
        pers = ctx.enter_context(tc.tile_pool(name="pers", bufs=1))

        # persistent constants (wqk loaded after first xT block below)
        wqk = pers.tile([128, 6 * 2 * DHC], BF16, tag="wqk", name="wqk")
        wv = pers.tile([128, 6 * DHC], BF16, tag="wv", name="wv")
        bqk = []
        if add_bqk:
            for m, (c0, msz) in enumerate(
                    ((0, 128), (128, 128), (256, 64), (320, 64))):
                bt = pers.tile([msz, 1], F32, tag=f"bqk{m}", name=f"bqk{m}")
                nc.sync.dma_start(bt[:], bqk_d[c0:c0 + msz, :])
                bqk.append(bt)
        t_ge = pers.tile([128, 128], BF16, tag="t_ge", name="t_ge")
        t_le = pers.tile([128, 128], BF16, tag="t_le", name="t_le")
        if add_bv:
            bvr = pers.tile([1, DHC], BF16, tag="bvr", name="bvr")
            nc.sync.dma_start(bvr[:], bvr_d)
            ones1 = pers.tile([1, 128], BF16, tag="ones1", name="ones1")
            nc.gpsimd.memset(ones1[:], 1.0)
        if use_fmask:
            fmk = pers.tile([128, NKT], F32, tag="fmk", name="fmk")
            nc.sync.dma_start(fmk[:], fmk_d)
        if use_qmask:
            qmk = pers.tile([128, NKT], F32, tag="qmk", name="qmk")
            nc.sync.dma_start(qmk[:], qmk_d)

        # persistent activations
        xTt = pers.tile([128, 6 * S], BF16, tag="xT", name="xT")
        xT3 = xTt.rearrange("p (a s) -> p a s", a=6)
        qkT = [pers.tile([128 if m < 2 else 64, S], BF16, tag=f"qkT{m}",
                         name=f"qkT{m}") for m in range(4)]
        va = pers.tile([128, NKT * HPC * VAW], BF16, tag="va", name="va")
        va4 = va.rearrange("p (t h c) -> p t h c", h=HPC, c=VAW)
        nc.gpsimd.memset(va4[:, :, :, DH:VAW], 1.0)

        # qkT layout: t0=[q0|q1], t1=[k0|k1], t2=q2, t3=k2 -> q_h and k_h
        # slices always share a partition base (matmul requirement)
        def q_slice(h):
            return (0, 64 * h) if h < 2 else (2, 0)
        def k_slice(h):
            return (1, 64 * h) if h < 2 else (3, 0)

        def emit_xT(sb):
            s0 = sb * SBLK
            nc.sync.dma_start(
                xT3[:, :, s0:s0 + SBLK],
                x_d[:, s0:s0 + SBLK].rearrange("(a p) s -> p a s", p=128))

        with tc.tile_pool(name="p2s", bufs=1) as p2s, \
             tc.tile_pool(name="pp_pj", bufs=2, space="PSUM") as pjp, \
             tc.tile_pool(name="pp_sc", bufs=1, space="PSUM") as scp, \
             tc.tile_pool(name="pp_av", bufs=1, space="PSUM") as avp:
            scb = scp.tile([128, 2 * 1280], F32, tag="scb", name="scb")
            sc_par = [0]

            def emit_proj_m(sb, m):
                s0 = sb * SBLK
                for m, (c0, msz) in ((m, ((0, 128), (128, 128), (256, 64),
                                          (320, 64))[m]),):
                    pj = pjp.tile([128, SBLK], F32, tag="pj", name="pj")
                    for kt in range(6):
                        nc.tensor.matmul(
                            pj[0:msz, :],
                            wqk[:, kt * 2 * DHC + c0:kt * 2 * DHC + c0 + msz],
                            xT3[:, kt, s0:s0 + SBLK],
                            start=(kt == 0), stop=(kt == 5))
                    if add_bqk:
                        nc.vector.tensor_scalar_add(
                            qkT[m][:, s0:s0 + SBLK], pj[0:msz, :], bqk[m][:])
                    else:
                        nc.vector.tensor_copy(qkT[m][:, s0:s0 + SBLK],
                                              pj[0:msz, :])

            def emit_v(st):
                pv = pjp.tile([128, SBLK], F32, tag="pj", name="pv")
                for kt in range(6):
                    nc.tensor.matmul(
                        pv[:, 0:DHC],
                        xT3[:, kt, st * 128:(st + 1) * 128],
                        wv[:, kt * DHC:(kt + 1) * DHC],
                        start=(kt == 0), stop=(kt == 5 and not add_bv))
                if add_bv:
                    nc.tensor.matmul(pv[:, 0:DHC], ones1[:], bvr[:],
                                     start=False, stop=True)
                nc.vector.tensor_copy(
                    va4[:, st, :, 0:DH],
                    pv[:, 0:DHC].rearrange("p (h d) -> p h d", h=HPC))

            def emit_masks(pt, blocks, masks):
                ge = sorted(c for c, w in masks if w == 'ge')
                le = sorted(c for c, w in masks if w == 'le')
                for cols, msk in ((ge, t_ge), (le, t_le)):
                    if len(cols) == 2:
                        base = pt[:, cols[0]:cols[0] + 128]
                        pstride = base.ap[0][0]
                        ppair = AP(base.tensor, base.offset,
                                   [[pstride, 128], [cols[1] - cols[0], 2],
                                    [1, 128]])
                        mpair = msk[:].unsqueeze(1).broadcast_to([128, 2, 128])
                        nc.vector.tensor_tensor(ppair, ppair, mpair,
                                                op=AluOp.mult)
                    else:
                        for c in cols:
                            nc.vector.tensor_tensor(
                                pt[:, c:c + 128], pt[:, c:c + 128], msk[:],
                                op=AluOp.mult)
                if use_fmask:
                    for kt, col, wd, qoff in blocks:
                        nc.vector.tensor_scalar_mul(
                            pt[:, col:col + wd], pt[:, col:col + wd],
                            fmk[:, kt:kt + 1])

            def attn_front(ci, heads):
                """scores -> exp -> masks for one or two heads of chunk ci.

                Two heads (h0,h1) sit at partition bases 0/64 of the same
                qkT tiles, so their K=64 score matmuls land in disjoint PE
                row groups and overlap when emitted adjacently. A pair
                claims both scb halves and runs a single fused Exp."""
                blocks, ncols, av_blocks, masks = _chunk_layout(ci)
                q0 = ci * C2
                pair = len(heads) == 2
                if pair:
                    scs = [scb[:, 0:1280], scb[:, 1280:2560]]
                else:
                    par = sc_par[0]
                    sc_par[0] ^= 1
                    scs = [scb[:, par * 1280:(par + 1) * 1280]]
                for kt, col, wd, qoff in blocks:
                    for h, sc in zip(heads, scs):
                        mq, rq = q_slice(h)
                        mk, rk = k_slice(h)
                        nc.tensor.matmul(
                            sc[:, col:col + wd],
                            qkT[mk][rk:rk + 64, kt * 128:(kt + 1) * 128],
                            qkT[mq][rq:rq + 64, q0 + qoff:q0 + qoff + wd],
                            start=True, stop=True)
                pts = []
                if pair:
                    pt2 = p2s.tile([128, 2560], BF16, tag="pt2", name="pt2",
                                   bufs=3)
                    nc.scalar.activation(pt2[:, 0:1280 + ncols],
                                         scb[:, 0:1280 + ncols], ActFn.Exp)
                    pts = [pt2[:, 0:1280], pt2[:, 1280:2560]]
                else:
                    pt = p2s.tile([128, 1280], BF16, tag="pt", name="pt",
                                  bufs=3)
                    nc.scalar.activation(pt[:, 0:ncols], scs[0][:, 0:ncols],
                                         ActFn.Exp)
                    pts = [pt]
                for pt in pts:
                    emit_masks(pt, blocks, masks)
                return pts, av_blocks

            def attn_back(ci, h, av, pt, av_blocks):
                for hf in range(2):
                    g = h * 2 + hf
                    lst = av_blocks[hf]
                    for i, (kt, col) in enumerate(lst):
                        nc.tensor.matmul(
                            av[:, g * VAW:g * VAW + VAW],
                            pt[:, col:col + 128], va4[:, kt, h, :],
                            start=(i == 0), stop=(i == len(lst) - 1))

            def epilogue(ci, av):
                av3 = av.rearrange("p (g c) -> p g c", c=VAW)
                rzs = p2s.tile([128, 6], F32, tag="rzs", name="rzs", bufs=3)
                nc.vector.reciprocal(rzs[:], av3[:, :, DH])
                if use_qmask:
                    for g in range(6):
                        nc.vector.tensor_scalar_mul(
                            rzs[:, g:g + 1], rzs[:, g:g + 1],
                            qmk[:, 2 * ci + (g % 2):2 * ci + (g % 2) + 1])
                for hf in range(2):
                    os_t = p2s.tile([128, DHC], F32, tag="os", name="os",
                                    bufs=4)
                    out3 = os_t.rearrange("p (h d) -> p h d", h=HPC)
                    nc.vector.tensor_tensor(
                        out3[:], av3[:, hf::2, 0:DH],
                        rzs[:, hf::2].broadcast_to([128, HPC, DH]),
                        op=AluOp.mult)
                    qt = 2 * ci + hf
                    nc.sync.dma_start(
                        out_d[qt * 128:(qt + 1) * 128, :], os_t[:])

            # ---- interleaved emission with 2-step software pipeline ----
            pending = []          # [(ci, h, av, pt, av_blocks)]
            av_cur = [None]       # av tile for current ci

            def push_step(ci, heads):
                pts, av_blocks = attn_front(ci, heads)
                if heads[0] == 0:
                    av_cur[0] = avp.tile([128, 6 * VAW], F32, tag="av",
                                         name="av")
                for h, pt in zip(heads, pts):
                    pending.append((ci, h, av_cur[0], pt, av_blocks))
                while len(pending) > 4:
                    pop_step()

            def pop_step():
                ci, h, av, pt, av_blocks = pending.pop(0)
                attn_back(ci, h, av, pt, av_blocks)
                if h == HPC - 1:
                    epilogue(ci, av)

            def merge(at, units):
                # spread units evenly between attn steps (pair counts double)
                wts = [len(hs) for _, hs in at]
                tot = sum(wts)
                if not at:
                    for u in units:
                        u()
                    return
                cum = 0
                taken = 0
                for (ci, hs), w in zip(at, wts):
                    push_step(ci, hs)
                    cum += w
                    tgt = (len(units) * cum) // tot
                    while taken < tgt:
                        units[taken]()
                        taken += 1

            wqk4 = wqk.rearrange("p (a n) -> p a n", a=6)
            wqk_d4 = wqk_d.rearrange("(a p) n -> p a n", p=128)
            for kt in range(6):
                nc.sync.dma_start(xT3[:, kt, 0:SBLK],
                                  x_d[kt * 128:(kt + 1) * 128, 0:SBLK])
                if kt == 0:
                    nc.sync.dma_start(wqk4[:, :, 0:128], wqk_d4[:, :, 0:128])
            nc.sync.dma_start(wqk4[:, :, 128:2 * DHC],
                              wqk_d4[:, :, 128:2 * DHC])
            nc.sync.dma_start(wv[:], wv_d.rearrange("(a p) n -> p a n", p=128))
            emit_xT(1)
            nc.sync.dma_start(t_ge[:], tge_d)
            nc.sync.dma_start(t_le[:], tle_d)
            for m in range(4):
                emit_proj_m(0, m)
            for st in range(4):
                emit_v(st)
            # slot sb: chunk 2sb (merged with proj(sb+1), which chunk 2sb+1's
            # scores need) then chunk 2sb+1 (merged with V(sb+1), which its
            # backs -- popping a slot later -- need)
            for sb in range(NSB):
                if sb + 2 <= NSB - 1:
                    emit_xT(sb + 2)
                if sb + 1 < NSB:
                    projs = [lambda m=m, sb=sb: emit_proj_m(sb + 1, m)
                             for m in range(4)]
                    vs = [lambda st=st: emit_v(st)
                          for st in range(4 * (sb + 1), 4 * (sb + 1) + 4)]
                else:
                    projs, vs = [], []
                merge([(2 * sb, hs) for hs in ((0, 1), (2,))], projs)
                merge([(2 * sb + 1, hs) for hs in ((0, 1), (2,))], vs)
            while pending:
                pop_step()

    nc.compile()
    return nc


_prog_cache = {}


def _get_program(use_fmask, use_qmask, add_bv, add_bqk):
    key = (use_fmask, use_qmask, add_bv, add_bqk)
    if key not in _prog_cache:
        _prog_cache[key] = _build_program(use_fmask, use_qmask, add_bv, add_bqk)
    return _prog_cache[key]


def _host_constants():
    kl = np.arange(128)[:, None]
    ql = np.arange(128)[None, :]
    t_ge = (kl >= ql).astype(ml_dtypes.bfloat16)
    t_le = (kl <= ql).astype(ml_dtypes.bfloat16)
    return t_ge, t_le


def kernel(hidden_states, attention_mask, is_index_masked, Wq, bq, Wk, bk, Wv, bv,
           trace=False):
    hidden_states = np.asarray(hidden_states, dtype=np.float32)
    attention_mask = np.asarray(attention_mask, dtype=np.float32)
    is_index_masked = np.asarray(is_index_masked)
    Wq = np.asarray(Wq, dtype=np.float32)
    Wk = np.asarray(Wk, dtype=np.float32)
    Wv = np.asarray(Wv, dtype=np.float32)
    bq = np.asarray(bq, dtype=np.float32)
    bk = np.asarray(bk, dtype=np.float32)
    bv = np.asarray(bv, dtype=np.float32)

    use_fmask = bool(np.any(attention_mask != 0))
    use_qmask = bool(np.any(is_index_masked))
    add_bv = bool(np.any(bv != 0))
    add_bqk = bool(np.any(bq != 0) or np.any(bk != 0))
    nc = _get_program(use_fmask, use_qmask, add_bv, add_bqk)

    scale = 1.0 / math.sqrt(DH)
    t_ge, t_le = _host_constants()
    xt16 = [np.ascontiguousarray(hidden_states[b].astype(ml_dtypes.bfloat16).T)
            for b in range(B)]

    in_maps = []
    for cid in range(NCORES):
        b = cid // 4
        h0 = HPC * (cid % 4)
        c0, c1 = h0 * DH, (h0 + HPC) * DH
        wqk = np.concatenate([
            Wq[:, c0:c0 + 128] * scale, Wk[:, c0:c0 + 128],
            Wq[:, c0 + 128:c1] * scale, Wk[:, c0 + 128:c1]], axis=1)
        m = {
            "xt16": xt16[b],
            "wqk": np.ascontiguousarray(wqk.astype(ml_dtypes.bfloat16)),
            "wv": np.ascontiguousarray(Wv[:, c0:c1].astype(ml_dtypes.bfloat16)),
            "t_ge": t_ge,
            "t_le": t_le,
        }
        if add_bqk:
            bqk = np.concatenate([
                bq[c0:c0 + 128] * scale, bk[c0:c0 + 128],
                bq[c0 + 128:c1] * scale, bk[c0 + 128:c1]])
            m["bqk"] = np.ascontiguousarray(bqk.reshape(2 * DHC, 1))
        if add_bv:
            m["bvrow"] = np.ascontiguousarray(
                bv[c0:c1].astype(ml_dtypes.bfloat16).reshape(1, DHC))
        if use_fmask:
            fac = (attention_mask[b] == 0).astype(np.float32)  # keep-factor
            m["fmk"] = np.ascontiguousarray(fac.reshape(NKT, 128).T)
        if use_qmask:
            keep = (~is_index_masked[b]).astype(np.float32)
            m["qmk"] = np.ascontiguousarray(keep.reshape(NKT, 128).T)
        in_maps.append(m)

    res = run_bass_kernel_spmd(nc, in_maps, core_ids=list(range(NCORES)),
                               trace=trace)
    out = np.empty((B, S, D), dtype=np.float32)
    for cid in range(NCORES):
        b = cid // 4
        h0 = HPC * (cid % 4)
        out[b, :, h0 * DH:(h0 + HPC) * DH] = res.results[cid]["out"]
    if trace:
        return out, res
    return out
